# revision 25
# baseline (speedup 1.0000x reference)
"""ComplexDenseSO2 Trainium2 kernel.

Computes out = (X @ conj(B)^T * w) @ B for complex X [64, 32400],
B [2048, 32400], w [2048], given as separate re/im fp32 planes.

Strategy (tensor-parallel over D across 8 cores):
  - Fold w into the first-matmul operand on the host:
    M = diag(w) @ conj(B), so mm1 output IS Y = X @ M^T.
  - Pad D 32400 -> 32768; core c owns d-slice [c*4096, (c+1)*4096).
  - mm1 runs k-chunk-OUTER (NKC=2 chunks of 1024 k-cols), all 32
    d-tiles inner, so chunk 0's coefficients finish at ~1/2 of the mt
    stream instead of at the end.  Per chunk: evacuate PSUM, PE-
    transpose to k-major, combine re/im into a [-Yi | Yr | Yi] layout
    (192 cols per k-block) so mm2's two stationaries are plain slices
    of the AllReduce output.
  - Collectives (5 total): a warm-up AllReduce at t~12us absorbs the
    ~11us CC cold-dispatch and boot skew; per chunk a 64-elem
    rendezvous AR (same gpsimd DMA queue as the arin write -> FIFO
    completion proves every core wrote arin, so the real AR never reads
    a half-written remote buffer) then the real 384KB AllReduce.  AR0
    overlaps mm1 chunk 1; AR1 overlaps the bn stream.
  - Queue discipline: mt+bn bulk slabs alternate sync/scalar queues
    with nothing blocking in front of them; xt rides scalar first;
    arin/dbar writes and AR-output reads ride the gpsimd queue.  The
    6-deep bn buffer keeps the bulk stream running while mm2 waits on
    a late AllReduce.  DMA stays ~saturated end to end (the kernel is
    DMA-roofline bound: ~67MB/core at ~358 GB/s).
  - mm2: moving tiles are host-packed [128, 8192] (Br|Bi) slabs
    consumed at DMA pace right behind the mt stream.  The 8 PSUM banks
    rotate mm1 accumulators -> transpose scratch -> mm2 accumulators
    via pool tags; epilogue descales and stores fp16.
  - fp16 operands use power-of-2 prescales (M*1024, B*256) to stay
    clear of fp16 subnormals; the epilogue descales by 2^-18.

Note: a faster coefficient reduction via direct SBUF->SBUF remote DMA
(remote_dma_broadcast, XOR-relative dests) was prototyped and works in
isolation on this runtime (see canary*.py), but this runtime crashes
when CC collectives and remote DMA sends run in the same window, and
it caps remote traffic at ~7 one-shot sends of <=1KB/row -- too little
to replace the AllReduce outright.
"""

import sys

if "/opt/trn_rl_repo" not in sys.path:
    sys.path.insert(0, "/opt/trn_rl_repo")

import numpy as np

B_, K, D = 64, 2048, 32400
NCORES = 8
DP = 32768
DL = DP // NCORES  # 4096

SCALE_M = 1024.0
SCALE_B = 256.0

NKC = 2            # mm1 k-chunks
KCW = K // NKC     # 1024 k columns per chunk
NDT = DL // 128    # 32 d-tiles
NKB = K // 128     # 16 k-blocks (mm2 stationaries)
KBC = KCW // 128   # 8 k-blocks per chunk
MSLB = 8           # mt slabs per chunk (4 d-tiles each)
DPC = NDT // MSLB  # 4 d-tiles per mt slab
YTW = 192          # [-Yi | Yr | Yi] cols per k-block (mm2 stationary)

_nc_cache = {}


def build_nc(n_cores=NCORES, k=K, dl=DL):
    import concourse.mybir as mybir
    from concourse import bacc
    import concourse.tile as tile
    from concourse.masks import make_identity

    fp = mybir.dt.float16
    f32 = mybir.dt.float32

    nc = bacc.Bacc(
        trn_type="TRN2",
        target_bir_lowering=False,
        debug=False,
        num_devices=n_cores,
    )
    # host-packed layouts (see _prep_in_maps)
    xt = nc.dram_tensor("xt", [128, dl], fp, kind="ExternalInput")
    mt = nc.dram_tensor(
        "mt", [NKC * MSLB * 128, DPC * 2 * KCW], fp, kind="ExternalInput"
    )
    bn = nc.dram_tensor("bn", [NKB * 128, 2 * dl], fp, kind="ExternalInput")
    out = nc.dram_tensor("out", [128, dl], fp, kind="ExternalOutput")

    SLBW = DPC * 2 * KCW       # 8192 cols per mt slab
    ARW = KBC * YTW            # 1536 cols: [-Yi | Yr | Yi] (mm2 stationary)

    with tile.TileContext(nc) as tc:
        with (
            tc.tile_pool(name="sb", bufs=2) as sb,
            tc.tile_pool(name="sbx", bufs=1) as sbx,
            tc.tile_pool(name="ps", bufs=1, space="PSUM") as ps,
            tc.tile_pool(name="dram", bufs=1, space="DRAM") as dram,
        ):
            ident = sbx.tile([128, 128], fp, tag="ident")
            make_identity(nc, ident)
            xts_all = sbx.tile([128, dl], fp, tag="xts_all", name="xts_all")
            nc.scalar.dma_start(out=xts_all, in_=xt.ap())
            xts = [xts_all[:, dt * 128 : (dt + 1) * 128] for dt in range(NDT)]

            # entry rendezvous: the CC AllReduce completes only after every
            # core triggered it, i.e. every core is executing (past NEFF-load
            # sem reset), so remote sem increments cannot be lost.  Also
            # warms the CC dispatch path.
            wup_in = dram.tile([1, 64], fp, tag="wup_in", name="wup_in")
            wup_out = dram.tile(
                [1, 64], fp, tag="wup_out", name="wup_out", addr_space="Shared"
            )
            nc.gpsimd.dma_start(out=wup_in, in_=ident[0:1, 0:64])
            nc.gpsimd.collective_compute(
                "AllReduce",
                mybir.AluOpType.add,
                ins=[wup_in.opt()],
                outs=[wup_out.opt()],
                replica_groups=[list(range(n_cores))],
            )
            arins, arouts, dbis, dbos = [], [], [], []
            for c in range(NKC):
                arins.append(
                    dram.tile([128, ARW], fp, tag=f"arin{c}", name=f"arin{c}")
                )
                arouts.append(
                    dram.tile(
                        [128, ARW], fp, tag=f"arout{c}", name=f"arout{c}",
                        addr_space="Shared",
                    )
                )
                dbis.append(
                    dram.tile([1, 64], fp, tag=f"dbi{c}", name=f"dbi{c}")
                )
                dbos.append(
                    dram.tile(
                        [1, 64], fp, tag=f"dbo{c}", name=f"dbo{c}",
                        addr_space="Shared",
                    )
                )

            # ---------------- mm1 (k-chunk outer, d inner) ----------------
            yts = []
            for c in range(NKC):
                # 4 single-bank accumulators per chunk (r/i x lo/hi 512)
                acc = [
                    ps.tile([128, 512], f32, tag=tg, name=f"a{tg}{c}", bufs=2)
                    for tg in ("pA", "pB", "pC", "pD")
                ]
                for s in range(MSLB):
                    m = c * MSLB + s
                    eng = nc.sync if m % 2 == 0 else nc.scalar
                    mt_t = sb.tile(
                        [128, SLBW], fp, tag="mtstream", name="mt", bufs=4
                    )
                    if m == 0:
                        # split the first slab so dt=0 matmuls start early
                        nc.sync.dma_start(
                            out=mt_t[:, 0 : SLBW // 2],
                            in_=mt[0:128, 0 : SLBW // 2],
                        )
                        nc.sync.dma_start(
                            out=mt_t[:, SLBW // 2 : SLBW],
                            in_=mt[0:128, SLBW // 2 : SLBW],
                        )
                    else:
                        eng.dma_start(
                            out=mt_t, in_=mt[m * 128 : (m + 1) * 128, :]
                        )
                    for dtl in range(DPC):
                        dt = s * DPC + dtl
                        st = dt == 0
                        sp = dt == NDT - 1
                        for q in range(4):
                            nc.tensor.matmul(
                                acc[q],
                                lhsT=xts[dt],
                                rhs=mt_t[
                                    :,
                                    dtl * 2 * KCW + q * 512 : dtl * 2 * KCW
                                    + (q + 1) * 512,
                                ],
                                start=st,
                                stop=sp,
                            )
                # evacuate PSUM -> fp16 SBUF (cast)
                a_r = sb.tile([128, KCW], fp, tag="a_r", name=f"a_r{c}", bufs=2)
                nc.vector.tensor_copy(a_r[:, 0:512], acc[0])
                nc.vector.tensor_copy(a_r[:, 512:1024], acc[1])
                a_i = sb.tile([128, KCW], fp, tag="a_i", name=f"a_i{c}", bufs=2)
                nc.vector.tensor_copy(a_i[:, 0:512], acc[2])
                nc.vector.tensor_copy(a_i[:, 512:1024], acc[3])
                # k-major via PE transposes into fp16 PSUM (reuses the pA/pC
                # banks this chunk just freed; same byte size as two acc
                # tiles, so the tag rotation lines up)
                tp_r = ps.tile([128, KCW], fp, tag="pA", name=f"tpr{c}", bufs=2)
                tp_i = ps.tile([128, KCW], fp, tag="pC", name=f"tpi{c}", bufs=2)
                for t in range(KCW // 128):
                    ts = slice(t * 128, (t + 1) * 128)
                    nc.tensor.transpose(tp_r[:, ts], a_r[:, ts], ident)
                    nc.tensor.transpose(tp_i[:, ts], a_i[:, ts], ident)
                # combine in k-major: j is now the free dim.  DVE may read
                # only one PSUM operand, so evacuate tp_r first.
                cc_r = sb.tile([128, KCW], fp, tag="cc_r", name=f"cc_r{c}", bufs=2)
                nc.vector.tensor_copy(cc_r, tp_r)
                # Yr = re(X@Mr^T) - im(X@Mi^T); Yi = im(X@Mr^T) + re(X@Mi^T)
                c_t = sb.tile([128, ARW], fp, tag="c_t", name=f"c_t{c}", bufs=2)
                c3 = c_t.rearrange("p (t j) -> p t j", j=YTW)
                r3 = cc_r.rearrange("p (t j) -> p t j", j=128)
                i3 = tp_i.rearrange("p (t j) -> p t j", j=128)
                nc.vector.tensor_sub(
                    c3[:, :, 64:128], r3[:, :, 0:64], i3[:, :, 64:128]
                )
                nc.vector.tensor_add(
                    c3[:, :, 128:192], r3[:, :, 64:128], i3[:, :, 0:64]
                )
                nc.vector.tensor_scalar_mul(
                    c3[:, :, 0:64], c3[:, :, 128:192], -1.0
                )
                nc.gpsimd.dma_start(out=arins[c], in_=c_t)

                # rendezvous: tiny AR on the same (gpsimd) queue as the arin
                # write; its completion implies every core finished writing
                # arin[c], so the real AllReduce never reads a half-written
                # remote buffer.
                dbar_sb = sb.tile([1, 64], fp, tag="dbs", name=f"dbs{c}", bufs=2)
                nc.vector.tensor_copy(dbar_sb, c_t[0:1, 0:64])
                nc.gpsimd.dma_start(out=dbis[c], in_=dbar_sb)
                nc.gpsimd.collective_compute(
                    "AllReduce",
                    mybir.AluOpType.add,
                    ins=[dbis[c].opt()],
                    outs=[dbos[c].opt()],
                    replica_groups=[list(range(n_cores))],
                )
                nc.gpsimd.collective_compute(
                    "AllReduce",
                    mybir.AluOpType.add,
                    ins=[arins[c].opt()],
                    outs=[arouts[c].opt()],
                    replica_groups=[list(range(n_cores))],
                )

            # AR-dependent reads, all on the gpsimd queue after the chunk
            # loop: a late AR stalls nothing except the matmuls that truly
            # need it (the 6-deep bn buffer keeps the bulk queues streaming).
            yts = []
            for c in range(NKC):
                ytA = sbx.tile([128, ARW], fp, tag=f"ytA{c}", name=f"ytA{c}")
                nc.gpsimd.dma_start(out=ytA, in_=arouts[c])
                yts.append(ytA)

            # ---------------- mm2 (DMA-paced, right behind mt) ----------------
            # po banks continue the pA..pD tag rotations (all 8 PSUM banks).
            po = [
                ps.tile([128, 512], f32, tag=tg, name=f"po{h}", bufs=2)
                for h, tg in enumerate(
                    ("pA", "pA", "pB", "pB", "pC", "pC", "pD", "pD")
                )
            ]
            for kb in range(NKB):
                c = kb // KBC
                eng = nc.sync if kb % 2 == 0 else nc.scalar
                bn_t = sb.tile(
                    [128, 2 * dl], fp, tag="bnstream", name="bn", bufs=6
                )
                eng.dma_start(out=bn_t, in_=bn[kb * 128 : (kb + 1) * 128, :])
                s0 = (kb % KBC) * YTW
                st, sp = kb == 0, kb == NKB - 1
                for h in range(8):
                    nc.tensor.matmul(
                        po[h],
                        lhsT=yts[c][:, s0 + 64 : s0 + 192],
                        rhs=bn_t[:, h * 512 : (h + 1) * 512],
                        start=st,
                        stop=False,
                    )
                for h in range(8):
                    nc.tensor.matmul(
                        po[h],
                        lhsT=yts[c][:, s0 : s0 + 128],
                        rhs=bn_t[:, dl + h * 512 : dl + (h + 1) * 512],
                        start=False,
                        stop=sp,
                    )
            for h in range(8):
                o_t = sb.tile([128, 512], fp, tag="o_t", name="o_t", bufs=4)
                nc.vector.tensor_scalar_mul(o_t, po[h], 1.0 / (SCALE_M * SCALE_B))
                eng = nc.sync if h % 2 == 0 else nc.gpsimd
                eng.dma_start(out=out[:, h * 512 : (h + 1) * 512], in_=o_t)

    nc.compile()
    return nc


def _get_nc(n_cores=NCORES, k=K, dl=DL):
    key = (n_cores, k, dl)
    if key not in _nc_cache:
        _nc_cache[key] = build_nc(n_cores, k, dl)
    return _nc_cache[key]


def _prep_in_maps(X_re, X_im, bases_re, bases_im, weight_re, weight_im):
    cdt = np.float16
    f32 = np.float32
    X_re = np.asarray(X_re, f32)
    X_im = np.asarray(X_im, f32)
    bases_re = np.asarray(bases_re, f32)
    bases_im = np.asarray(bases_im, f32)
    wr = np.asarray(weight_re, f32)[:, None]
    wi = np.asarray(weight_im, f32)[:, None]

    # M = diag(w) @ conj(B): Mr = wr*Br + wi*Bi ; Mi = wi*Br - wr*Bi
    mr = (wr * bases_re + wi * bases_im) * np.float32(SCALE_M)
    mi = (wi * bases_re - wr * bases_im) * np.float32(SCALE_M)
    bsr = (bases_re * np.float32(SCALE_B)).astype(cdt)
    bsi = (bases_im * np.float32(SCALE_B)).astype(cdt)
    mr = mr.astype(cdt)
    mi = mi.astype(cdt)

    in_maps = []
    for c in range(NCORES):
        lo = c * DL
        hi = min((c + 1) * DL, D)
        n = hi - lo

        # xt[p, dt*128 + j] = Xstack^T[dt*128+p, j], j: 0:64 re, 64:128 im
        xtd = np.zeros((DL, 128), cdt)
        xtd[:n, 0:64] = X_re[:, lo:hi].T.astype(cdt)
        xtd[:n, 64:128] = X_im[:, lo:hi].T.astype(cdt)
        xt = (
            xtd.reshape(NDT, 128, 128).transpose(1, 0, 2).reshape(128, DL)
        )

        # mt slab (kc, s): rows p=d-within-tile, cols dtl*2*KCW + [Mr | Mi]
        # for k-chunk kc, d-tile dt = s*DPC + dtl.
        mrT = np.zeros((DL, K), cdt)
        miT = np.zeros((DL, K), cdt)
        mrT[:n, :] = mr[:, lo:hi].T
        miT[:n, :] = mi[:, lo:hi].T
        # r4[dt, p, kc, q]
        r4 = mrT.reshape(NDT, 128, NKC, KCW)
        i4 = miT.reshape(NDT, 128, NKC, KCW)
        mt = np.empty((NKC, MSLB, 128, DPC, 2, KCW), cdt)
        # -> [kc, s, p, dtl, plane, q]
        mt[:, :, :, :, 0, :] = (
            r4.transpose(2, 0, 1, 3)
            .reshape(NKC, MSLB, DPC, 128, KCW)
            .transpose(0, 1, 3, 2, 4)
        )
        mt[:, :, :, :, 1, :] = (
            i4.transpose(2, 0, 1, 3)
            .reshape(NKC, MSLB, DPC, 128, KCW)
            .transpose(0, 1, 3, 2, 4)
        )
        mt = mt.reshape(NKC * MSLB * 128, DPC * 2 * KCW)

        # bn[kb*128 + p, :] = [Br[k, d-shard] | Bi[k, d-shard]]
        bnd = np.zeros((K, 2 * DL), cdt)
        bnd[:, 0:n] = bsr[:, lo:hi]
        bnd[:, DL : DL + n] = bsi[:, lo:hi]

        in_maps.append({"xt": xt, "mt": mt, "bn": bnd})
    return in_maps


def run(inputs, trace=False, trace_kwargs=None):
    """Returns (full complex64 output [64, 32400], BassKernelResults)."""
    from concourse.bass_utils import run_bass_kernel_spmd

    in_maps = _prep_in_maps(**inputs)
    nc = _get_nc()
    res = run_bass_kernel_spmd(
        nc,
        in_maps,
        core_ids=list(range(NCORES)),
        trace=trace,
        **(trace_kwargs or {}),
    )
    parts = []
    for c in range(NCORES):
        o = res.results[c]["out"].astype(np.float32)
        parts.append(o[0:64, :] + 1j * o[64:128, :].astype(np.complex64))
    full = np.concatenate(parts, axis=1)[:, :D].astype(np.complex64)
    return full, res


def kernel(**inputs) -> np.ndarray:
    out, _ = run(inputs, trace=False)
    return out


# revision 26
# speedup vs baseline: 1.0148x; 1.0148x over previous
"""ComplexDenseSO2 Trainium2 kernel.

Computes out = (X @ conj(B)^T * w) @ B for complex X [64, 32400],
B [2048, 32400], w [2048], given as separate re/im fp32 planes.

Strategy (tensor-parallel over D across 8 cores):
  - Fold w into the first-matmul operand on the host:
    M = diag(w) @ conj(B), so mm1 output IS Y = X @ M^T.
  - Pad D 32400 -> 32768; core c owns d-slice [c*4096, (c+1)*4096).
  - mm1 runs k-chunk-OUTER (NKC=2 chunks of 1024 k-cols), all 32
    d-tiles inner, so chunk 0's coefficients finish at ~1/2 of the mt
    stream instead of at the end.  Per chunk: evacuate PSUM, PE-
    transpose to k-major, combine re/im into a [-Yi | Yr | Yi] layout
    (192 cols per k-block) so mm2's two stationaries are plain slices
    of the AllReduce output.
  - Collectives (5 total): a warm-up AllReduce at t~12us absorbs the
    ~11us CC cold-dispatch and boot skew; per chunk a 64-elem
    rendezvous AR (same gpsimd DMA queue as the arin write -> FIFO
    completion proves every core wrote arin, so the real AR never reads
    a half-written remote buffer) then the real 384KB AllReduce.  AR0
    overlaps mm1 chunk 1; AR1 overlaps the bn stream.
  - Queue discipline: mt+bn bulk slabs alternate sync/scalar queues
    with nothing blocking in front of them; xt rides scalar first;
    arin/dbar writes and AR-output reads ride the gpsimd queue.  The
    6-deep bn buffer keeps the bulk stream running while mm2 waits on
    a late AllReduce.  DMA stays ~saturated end to end (the kernel is
    DMA-roofline bound: ~67MB/core at ~358 GB/s).
  - mm2: moving tiles are host-packed [128, 8192] (Br|Bi) slabs
    consumed at DMA pace right behind the mt stream.  The 8 PSUM banks
    rotate mm1 accumulators -> transpose scratch -> mm2 accumulators
    via pool tags; epilogue descales and stores fp16.
  - fp16 operands use power-of-2 prescales (M*1024, B*256) to stay
    clear of fp16 subnormals; the epilogue descales by 2^-18.

Note: a faster coefficient reduction via direct SBUF->SBUF remote DMA
(remote_dma_broadcast, XOR-relative dests) was prototyped and works in
isolation on this runtime (see canary*.py), but this runtime crashes
when CC collectives and remote DMA sends run in the same window, and
it caps remote traffic at ~7 one-shot sends of <=1KB/row -- too little
to replace the AllReduce outright.
"""

import sys

if "/opt/trn_rl_repo" not in sys.path:
    sys.path.insert(0, "/opt/trn_rl_repo")

import numpy as np

B_, K, D = 64, 2048, 32400
NCORES = 8
DP = 32768
DL = DP // NCORES  # 4096

SCALE_M = 1024.0
SCALE_B = 256.0

NKC = 2            # mm1 k-chunks
KCW = K // NKC     # 1024 k columns per chunk
NDT = DL // 128    # 32 d-tiles
NKB = K // 128     # 16 k-blocks (mm2 stationaries)
KBC = KCW // 128   # 8 k-blocks per chunk
MSLB = 8           # mt slabs per chunk (4 d-tiles each)
DPC = NDT // MSLB  # 4 d-tiles per mt slab
YTW = 192          # [-Yi | Yr | Yi] cols per k-block (mm2 stationary)

_nc_cache = {}


def build_nc(n_cores=NCORES, k=K, dl=DL):
    import concourse.mybir as mybir
    from concourse import bacc
    import concourse.tile as tile
    from concourse.masks import make_identity

    fp = mybir.dt.float16
    f32 = mybir.dt.float32

    nc = bacc.Bacc(
        trn_type="TRN2",
        target_bir_lowering=False,
        debug=False,
        num_devices=n_cores,
    )
    # host-packed layouts (see _prep_in_maps)
    xt = nc.dram_tensor("xt", [128, dl], fp, kind="ExternalInput")
    mt = nc.dram_tensor(
        "mt", [NKC * MSLB * 128, DPC * 2 * KCW], fp, kind="ExternalInput"
    )
    bn = nc.dram_tensor("bn", [NKB * 128, 2 * dl], fp, kind="ExternalInput")
    out = nc.dram_tensor("out", [128, dl], fp, kind="ExternalOutput")

    SLBW = DPC * 2 * KCW       # 8192 cols per mt slab
    ARW = KBC * YTW            # 1536 cols: [-Yi | Yr | Yi] (mm2 stationary)

    with tile.TileContext(nc) as tc:
        with (
            tc.tile_pool(name="sb", bufs=2) as sb,
            tc.tile_pool(name="sbx", bufs=1) as sbx,
            tc.tile_pool(name="ps", bufs=1, space="PSUM") as ps,
            tc.tile_pool(name="dram", bufs=1, space="DRAM") as dram,
        ):
            ident = sbx.tile([128, 128], fp, tag="ident")
            make_identity(nc, ident)
            xts_all = sbx.tile([128, dl], fp, tag="xts_all", name="xts_all")
            nc.scalar.dma_start(out=xts_all, in_=xt.ap())
            xts = [xts_all[:, dt * 128 : (dt + 1) * 128] for dt in range(NDT)]

            # entry rendezvous: the CC AllReduce completes only after every
            # core triggered it, i.e. every core is executing (past NEFF-load
            # sem reset), so remote sem increments cannot be lost.  Also
            # warms the CC dispatch path.
            wup_in = dram.tile([1, 64], fp, tag="wup_in", name="wup_in")
            wup_out = dram.tile(
                [1, 64], fp, tag="wup_out", name="wup_out", addr_space="Shared"
            )
            nc.gpsimd.dma_start(out=wup_in, in_=ident[0:1, 0:64])
            nc.gpsimd.collective_compute(
                "AllReduce",
                mybir.AluOpType.add,
                ins=[wup_in.opt()],
                outs=[wup_out.opt()],
                replica_groups=[list(range(n_cores))],
            )
            arins, arouts, dbis, dbos = [], [], [], []
            for c in range(NKC):
                arins.append(
                    dram.tile([128, ARW], fp, tag=f"arin{c}", name=f"arin{c}")
                )
                arouts.append(
                    dram.tile(
                        [128, ARW], fp, tag=f"arout{c}", name=f"arout{c}",
                        addr_space="Shared",
                    )
                )
                dbis.append(
                    dram.tile([1, 64], fp, tag=f"dbi{c}", name=f"dbi{c}")
                )
                dbos.append(
                    dram.tile(
                        [1, 64], fp, tag=f"dbo{c}", name=f"dbo{c}",
                        addr_space="Shared",
                    )
                )

            # ---------------- mm1 (k-chunk outer, d inner) ----------------
            yts = []
            for c in range(NKC):
                # 4 single-bank accumulators per chunk (r/i x lo/hi 512)
                acc = [
                    ps.tile([128, 512], f32, tag=tg, name=f"a{tg}{c}", bufs=2)
                    for tg in ("pA", "pB", "pC", "pD")
                ]
                for s in range(MSLB):
                    m = c * MSLB + s
                    eng = nc.sync if m % 2 == 0 else nc.scalar
                    mt_t = sb.tile(
                        [128, SLBW], fp, tag="mtstream", name="mt", bufs=4
                    )
                    if m == 0:
                        # split the first slab so dt=0 matmuls start early
                        nc.sync.dma_start(
                            out=mt_t[:, 0 : SLBW // 2],
                            in_=mt[0:128, 0 : SLBW // 2],
                        )
                        nc.sync.dma_start(
                            out=mt_t[:, SLBW // 2 : SLBW],
                            in_=mt[0:128, SLBW // 2 : SLBW],
                        )
                    else:
                        eng.dma_start(
                            out=mt_t, in_=mt[m * 128 : (m + 1) * 128, :]
                        )
                    for dtl in range(DPC):
                        dt = s * DPC + dtl
                        st = dt == 0
                        sp = dt == NDT - 1
                        for q in range(4):
                            nc.tensor.matmul(
                                acc[q],
                                lhsT=xts[dt],
                                rhs=mt_t[
                                    :,
                                    dtl * 2 * KCW + q * 512 : dtl * 2 * KCW
                                    + (q + 1) * 512,
                                ],
                                start=st,
                                stop=sp,
                            )
                # evacuate PSUM -> fp16 SBUF (cast)
                a_r = sb.tile([128, KCW], fp, tag="a_r", name=f"a_r{c}", bufs=2)
                nc.vector.tensor_copy(a_r[:, 0:512], acc[0])
                nc.vector.tensor_copy(a_r[:, 512:1024], acc[1])
                a_i = sb.tile([128, KCW], fp, tag="a_i", name=f"a_i{c}", bufs=2)
                nc.vector.tensor_copy(a_i[:, 0:512], acc[2])
                nc.vector.tensor_copy(a_i[:, 512:1024], acc[3])
                # k-major via PE transposes into fp16 PSUM (reuses the pA/pC
                # banks this chunk just freed; same byte size as two acc
                # tiles, so the tag rotation lines up)
                tp_r = ps.tile([128, KCW], fp, tag="pA", name=f"tpr{c}", bufs=2)
                tp_i = ps.tile([128, KCW], fp, tag="pC", name=f"tpi{c}", bufs=2)
                for t in range(KCW // 128):
                    ts = slice(t * 128, (t + 1) * 128)
                    nc.tensor.transpose(tp_r[:, ts], a_r[:, ts], ident)
                    nc.tensor.transpose(tp_i[:, ts], a_i[:, ts], ident)
                # combine in k-major: j is now the free dim.  DVE may read
                # only one PSUM operand, so evacuate tp_r first.
                cc_r = sb.tile([128, KCW], fp, tag="cc_r", name=f"cc_r{c}", bufs=2)
                nc.vector.tensor_copy(cc_r, tp_r)
                # Yr = re(X@Mr^T) - im(X@Mi^T); Yi = im(X@Mr^T) + re(X@Mi^T)
                c_t = sb.tile([128, ARW], fp, tag="c_t", name=f"c_t{c}", bufs=2)
                c3 = c_t.rearrange("p (t j) -> p t j", j=YTW)
                r3 = cc_r.rearrange("p (t j) -> p t j", j=128)
                i3 = tp_i.rearrange("p (t j) -> p t j", j=128)
                nc.vector.tensor_sub(
                    c3[:, :, 64:128], r3[:, :, 0:64], i3[:, :, 64:128]
                )
                nc.vector.tensor_add(
                    c3[:, :, 128:192], r3[:, :, 64:128], i3[:, :, 0:64]
                )
                nc.vector.tensor_scalar_mul(
                    c3[:, :, 0:64], c3[:, :, 128:192], -1.0
                )
                nc.gpsimd.dma_start(out=arins[c], in_=c_t)

                # rendezvous before AR0: its completion implies every core
                # finished writing arin[0], so the real AllReduce never reads
                # a half-written remote buffer.  AR1 needs no rendezvous: the
                # CC runs collectives serially, so AR1's mesh starts only
                # after AR0 completed on every core -- by which point every
                # core wrote arin[1] long ago (it precedes the AR1 trigger,
                # and AR0's completion certifies global progress).
                if c == 0:
                    dbar_sb = sb.tile([1, 64], fp, tag="dbs", name=f"dbs{c}", bufs=2)
                    nc.vector.tensor_copy(dbar_sb, c_t[0:1, 0:64])
                    nc.gpsimd.dma_start(out=dbis[c], in_=dbar_sb)
                    nc.gpsimd.collective_compute(
                        "AllReduce",
                        mybir.AluOpType.add,
                        ins=[dbis[c].opt()],
                        outs=[dbos[c].opt()],
                        replica_groups=[list(range(n_cores))],
                    )
                nc.gpsimd.collective_compute(
                    "AllReduce",
                    mybir.AluOpType.add,
                    ins=[arins[c].opt()],
                    outs=[arouts[c].opt()],
                    replica_groups=[list(range(n_cores))],
                )

            # AR-dependent reads, all on the gpsimd queue after the chunk
            # loop: a late AR stalls nothing except the matmuls that truly
            # need it (the 6-deep bn buffer keeps the bulk queues streaming).
            yts = []
            for c in range(NKC):
                ytA = sbx.tile([128, ARW], fp, tag=f"ytA{c}", name=f"ytA{c}")
                nc.gpsimd.dma_start(out=ytA, in_=arouts[c])
                yts.append(ytA)

            # ---------------- mm2 (DMA-paced, right behind mt) ----------------
            # po banks continue the pA..pD tag rotations (all 8 PSUM banks).
            po = [
                ps.tile([128, 512], f32, tag=tg, name=f"po{h}", bufs=2)
                for h, tg in enumerate(
                    ("pA", "pA", "pB", "pB", "pC", "pC", "pD", "pD")
                )
            ]
            for kb in range(NKB):
                c = kb // KBC
                eng = nc.sync if kb % 2 == 0 else nc.scalar
                bn_t = sb.tile(
                    [128, 2 * dl], fp, tag="bnstream", name="bn", bufs=6
                )
                eng.dma_start(out=bn_t, in_=bn[kb * 128 : (kb + 1) * 128, :])
                s0 = (kb % KBC) * YTW
                st, sp = kb == 0, kb == NKB - 1
                for h in range(8):
                    nc.tensor.matmul(
                        po[h],
                        lhsT=yts[c][:, s0 + 64 : s0 + 192],
                        rhs=bn_t[:, h * 512 : (h + 1) * 512],
                        start=st,
                        stop=False,
                    )
                for h in range(8):
                    nc.tensor.matmul(
                        po[h],
                        lhsT=yts[c][:, s0 : s0 + 128],
                        rhs=bn_t[:, dl + h * 512 : dl + (h + 1) * 512],
                        start=False,
                        stop=sp,
                    )
            for h in range(8):
                o_t = sb.tile([128, 512], fp, tag="o_t", name="o_t", bufs=4)
                nc.vector.tensor_scalar_mul(o_t, po[h], 1.0 / (SCALE_M * SCALE_B))
                eng = nc.sync if h % 2 == 0 else nc.gpsimd
                eng.dma_start(out=out[:, h * 512 : (h + 1) * 512], in_=o_t)

    nc.compile()
    return nc


def _get_nc(n_cores=NCORES, k=K, dl=DL):
    key = (n_cores, k, dl)
    if key not in _nc_cache:
        _nc_cache[key] = build_nc(n_cores, k, dl)
    return _nc_cache[key]


def _prep_in_maps(X_re, X_im, bases_re, bases_im, weight_re, weight_im):
    cdt = np.float16
    f32 = np.float32
    X_re = np.asarray(X_re, f32)
    X_im = np.asarray(X_im, f32)
    bases_re = np.asarray(bases_re, f32)
    bases_im = np.asarray(bases_im, f32)
    wr = np.asarray(weight_re, f32)[:, None]
    wi = np.asarray(weight_im, f32)[:, None]

    # M = diag(w) @ conj(B): Mr = wr*Br + wi*Bi ; Mi = wi*Br - wr*Bi
    mr = (wr * bases_re + wi * bases_im) * np.float32(SCALE_M)
    mi = (wi * bases_re - wr * bases_im) * np.float32(SCALE_M)
    bsr = (bases_re * np.float32(SCALE_B)).astype(cdt)
    bsi = (bases_im * np.float32(SCALE_B)).astype(cdt)
    mr = mr.astype(cdt)
    mi = mi.astype(cdt)

    in_maps = []
    for c in range(NCORES):
        lo = c * DL
        hi = min((c + 1) * DL, D)
        n = hi - lo

        # xt[p, dt*128 + j] = Xstack^T[dt*128+p, j], j: 0:64 re, 64:128 im
        xtd = np.zeros((DL, 128), cdt)
        xtd[:n, 0:64] = X_re[:, lo:hi].T.astype(cdt)
        xtd[:n, 64:128] = X_im[:, lo:hi].T.astype(cdt)
        xt = (
            xtd.reshape(NDT, 128, 128).transpose(1, 0, 2).reshape(128, DL)
        )

        # mt slab (kc, s): rows p=d-within-tile, cols dtl*2*KCW + [Mr | Mi]
        # for k-chunk kc, d-tile dt = s*DPC + dtl.
        mrT = np.zeros((DL, K), cdt)
        miT = np.zeros((DL, K), cdt)
        mrT[:n, :] = mr[:, lo:hi].T
        miT[:n, :] = mi[:, lo:hi].T
        # r4[dt, p, kc, q]
        r4 = mrT.reshape(NDT, 128, NKC, KCW)
        i4 = miT.reshape(NDT, 128, NKC, KCW)
        mt = np.empty((NKC, MSLB, 128, DPC, 2, KCW), cdt)
        # -> [kc, s, p, dtl, plane, q]
        mt[:, :, :, :, 0, :] = (
            r4.transpose(2, 0, 1, 3)
            .reshape(NKC, MSLB, DPC, 128, KCW)
            .transpose(0, 1, 3, 2, 4)
        )
        mt[:, :, :, :, 1, :] = (
            i4.transpose(2, 0, 1, 3)
            .reshape(NKC, MSLB, DPC, 128, KCW)
            .transpose(0, 1, 3, 2, 4)
        )
        mt = mt.reshape(NKC * MSLB * 128, DPC * 2 * KCW)

        # bn[kb*128 + p, :] = [Br[k, d-shard] | Bi[k, d-shard]]
        bnd = np.zeros((K, 2 * DL), cdt)
        bnd[:, 0:n] = bsr[:, lo:hi]
        bnd[:, DL : DL + n] = bsi[:, lo:hi]

        in_maps.append({"xt": xt, "mt": mt, "bn": bnd})
    return in_maps


def run(inputs, trace=False, trace_kwargs=None):
    """Returns (full complex64 output [64, 32400], BassKernelResults)."""
    from concourse.bass_utils import run_bass_kernel_spmd

    in_maps = _prep_in_maps(**inputs)
    nc = _get_nc()
    res = run_bass_kernel_spmd(
        nc,
        in_maps,
        core_ids=list(range(NCORES)),
        trace=trace,
        **(trace_kwargs or {}),
    )
    parts = []
    for c in range(NCORES):
        o = res.results[c]["out"].astype(np.float32)
        parts.append(o[0:64, :] + 1j * o[64:128, :].astype(np.complex64))
    full = np.concatenate(parts, axis=1)[:, :D].astype(np.complex64)
    return full, res


def kernel(**inputs) -> np.ndarray:
    out, _ = run(inputs, trace=False)
    return out


# revision 27
# speedup vs baseline: 1.0499x; 1.0346x over previous
"""ComplexDenseSO2 Trainium2 kernel.

Computes out = (X @ conj(B)^T * w) @ B for complex X [64, 32400],
B [2048, 32400], w [2048], given as separate re/im fp32 planes.

Strategy (tensor-parallel over D across 8 cores):
  - Fold w into the first-matmul operand on the host:
    M = diag(w) @ conj(B), so mm1 output IS Y = X @ M^T.
  - Pad D 32400 -> 32768; core c owns d-slice [c*4096, (c+1)*4096).
  - mm1 runs k-chunk-OUTER (NKC=2 chunks of 1024 k-cols), all 32
    d-tiles inner, so chunk 0's coefficients finish at ~1/2 of the mt
    stream instead of at the end.  Per chunk: evacuate PSUM, PE-
    transpose to k-major, combine re/im into a [-Yi | Yr | Yi] layout
    (192 cols per k-block) so mm2's two stationaries are plain slices
    of the AllReduce output.
  - Collectives (5 total): a warm-up AllReduce at t~12us absorbs the
    ~11us CC cold-dispatch and boot skew; per chunk a 64-elem
    rendezvous AR (same gpsimd DMA queue as the arin write -> FIFO
    completion proves every core wrote arin, so the real AR never reads
    a half-written remote buffer) then the real 384KB AllReduce.  AR0
    overlaps mm1 chunk 1; AR1 overlaps the bn stream.
  - Queue discipline: mt+bn bulk slabs alternate sync/scalar queues
    with nothing blocking in front of them; xt rides scalar first;
    arin/dbar writes and AR-output reads ride the gpsimd queue.  The
    6-deep bn buffer keeps the bulk stream running while mm2 waits on
    a late AllReduce.  DMA stays ~saturated end to end (the kernel is
    DMA-roofline bound: ~67MB/core at ~358 GB/s).
  - mm2: moving tiles are host-packed [128, 8192] (Br|Bi) slabs
    consumed at DMA pace right behind the mt stream.  The 8 PSUM banks
    rotate mm1 accumulators -> transpose scratch -> mm2 accumulators
    via pool tags; epilogue descales and stores fp16.
  - fp16 operands use power-of-2 prescales (M*1024, B*256) to stay
    clear of fp16 subnormals; the epilogue descales by 2^-18.

Note: a faster coefficient reduction via direct SBUF->SBUF remote DMA
(remote_dma_broadcast, XOR-relative dests) was prototyped and works in
isolation on this runtime (see canary*.py), but this runtime crashes
when CC collectives and remote DMA sends run in the same window, and
it caps remote traffic at ~7 one-shot sends of <=1KB/row -- too little
to replace the AllReduce outright.
"""

import sys

if "/opt/trn_rl_repo" not in sys.path:
    sys.path.insert(0, "/opt/trn_rl_repo")

import numpy as np

B_, K, D = 64, 2048, 32400
NCORES = 8
DP = 32768
DL = DP // NCORES  # 4096

SCALE_M = 1024.0
SCALE_B = 256.0

NKC = 2            # mm1 k-chunks
KCW = K // NKC     # 1024 k columns per chunk
NDT = DL // 128    # 32 d-tiles
NKB = K // 128     # 16 k-blocks (mm2 stationaries)
KBC = KCW // 128   # 8 k-blocks per chunk
MSLB = 8           # mt slabs per chunk (4 d-tiles each)
DPC = NDT // MSLB  # 4 d-tiles per mt slab
YTW = 192          # [-Yi | Yr | Yi] cols per k-block (mm2 stationary)

_nc_cache = {}


def build_nc(n_cores=NCORES, k=K, dl=DL):
    import concourse.mybir as mybir
    from concourse import bacc
    import concourse.tile as tile
    from concourse.masks import make_identity

    fp = mybir.dt.float16
    f32 = mybir.dt.float32

    nc = bacc.Bacc(
        trn_type="TRN2",
        target_bir_lowering=False,
        debug=False,
        num_devices=n_cores,
    )
    # host-packed layouts (see _prep_in_maps)
    xt = nc.dram_tensor("xt", [128, dl], fp, kind="ExternalInput")
    mt = nc.dram_tensor(
        "mt", [NKC * MSLB * 128, DPC * 2 * KCW], fp, kind="ExternalInput"
    )
    bn = nc.dram_tensor("bn", [NKB * 128, 2 * dl], fp, kind="ExternalInput")
    out = nc.dram_tensor("out", [128, dl], fp, kind="ExternalOutput")

    SLBW = DPC * 2 * KCW       # 8192 cols per mt slab
    ARW = KBC * YTW            # 1536 cols: [-Yi | Yr | Yi] (mm2 stationary)

    with tile.TileContext(nc) as tc:
        with (
            tc.tile_pool(name="sb", bufs=2) as sb,
            tc.tile_pool(name="sbx", bufs=1) as sbx,
            tc.tile_pool(name="ps", bufs=1, space="PSUM") as ps,
            tc.tile_pool(name="dram", bufs=1, space="DRAM") as dram,
        ):
            ident = sbx.tile([128, 128], fp, tag="ident")
            make_identity(nc, ident)
            xts_all = sbx.tile([128, dl], fp, tag="xts_all", name="xts_all")
            nc.scalar.dma_start(out=xts_all, in_=xt.ap())
            xts = [xts_all[:, dt * 128 : (dt + 1) * 128] for dt in range(NDT)]

            # entry rendezvous: the CC AllReduce completes only after every
            # core triggered it, i.e. every core is executing (past NEFF-load
            # sem reset), so remote sem increments cannot be lost.  Also
            # warms the CC dispatch path.
            wup_in = dram.tile([1, 64], fp, tag="wup_in", name="wup_in")
            wup_out = dram.tile(
                [1, 64], fp, tag="wup_out", name="wup_out", addr_space="Shared"
            )
            nc.gpsimd.dma_start(out=wup_in, in_=ident[0:1, 0:64])
            nc.gpsimd.collective_compute(
                "AllReduce",
                mybir.AluOpType.add,
                ins=[wup_in.opt()],
                outs=[wup_out.opt()],
                replica_groups=[list(range(n_cores))],
            )
            arins, arouts, dbis, dbos = [], [], [], []
            for c in range(NKC):
                arins.append(
                    dram.tile([128, ARW], fp, tag=f"arin{c}", name=f"arin{c}")
                )
                arouts.append(
                    dram.tile(
                        [128, ARW], fp, tag=f"arout{c}", name=f"arout{c}",
                        addr_space="Shared",
                    )
                )
                dbis.append(
                    dram.tile([1, 64], fp, tag=f"dbi{c}", name=f"dbi{c}")
                )
                dbos.append(
                    dram.tile(
                        [1, 64], fp, tag=f"dbo{c}", name=f"dbo{c}",
                        addr_space="Shared",
                    )
                )

            # ---------------- mm1 (k-chunk outer, d inner) ----------------
            yts = []
            for c in range(NKC):
                # 4 single-bank accumulators per chunk (r/i x lo/hi 512)
                acc = [
                    ps.tile([128, 512], f32, tag=tg, name=f"a{tg}{c}", bufs=2)
                    for tg in ("pA", "pB", "pC", "pD")
                ]
                for s in range(MSLB):
                    m = c * MSLB + s
                    eng = nc.sync if m % 2 == 0 else nc.scalar
                    mt_t = sb.tile(
                        [128, SLBW], fp, tag="mtstream", name="mt", bufs=4
                    )
                    if m == 0:
                        # split the first slab so dt=0 matmuls start early
                        nc.sync.dma_start(
                            out=mt_t[:, 0 : SLBW // 2],
                            in_=mt[0:128, 0 : SLBW // 2],
                        )
                        nc.sync.dma_start(
                            out=mt_t[:, SLBW // 2 : SLBW],
                            in_=mt[0:128, SLBW // 2 : SLBW],
                        )
                    else:
                        eng.dma_start(
                            out=mt_t, in_=mt[m * 128 : (m + 1) * 128, :]
                        )
                    for dtl in range(DPC):
                        dt = s * DPC + dtl
                        st = dt == 0
                        sp = dt == NDT - 1
                        for q in range(4):
                            nc.tensor.matmul(
                                acc[q],
                                lhsT=xts[dt],
                                rhs=mt_t[
                                    :,
                                    dtl * 2 * KCW + q * 512 : dtl * 2 * KCW
                                    + (q + 1) * 512,
                                ],
                                start=st,
                                stop=sp,
                            )
                # evacuate PSUM -> fp16 SBUF (cast)
                a_r = sb.tile([128, KCW], fp, tag="a_r", name=f"a_r{c}", bufs=2)
                nc.vector.tensor_copy(a_r[:, 0:512], acc[0])
                nc.vector.tensor_copy(a_r[:, 512:1024], acc[1])
                a_i = sb.tile([128, KCW], fp, tag="a_i", name=f"a_i{c}", bufs=2)
                nc.vector.tensor_copy(a_i[:, 0:512], acc[2])
                nc.vector.tensor_copy(a_i[:, 512:1024], acc[3])
                # k-major via PE transposes into fp16 PSUM (reuses the pA/pC
                # banks this chunk just freed; same byte size as two acc
                # tiles, so the tag rotation lines up)
                tp_r = ps.tile([128, KCW], fp, tag="pA", name=f"tpr{c}", bufs=2)
                tp_i = ps.tile([128, KCW], fp, tag="pC", name=f"tpi{c}", bufs=2)
                for t in range(KCW // 128):
                    ts = slice(t * 128, (t + 1) * 128)
                    nc.tensor.transpose(tp_r[:, ts], a_r[:, ts], ident)
                    nc.tensor.transpose(tp_i[:, ts], a_i[:, ts], ident)
                # combine in k-major: j is now the free dim.  DVE may read
                # only one PSUM operand, so evacuate tp_r first.
                cc_r = sb.tile([128, KCW], fp, tag="cc_r", name=f"cc_r{c}", bufs=2)
                nc.vector.tensor_copy(cc_r, tp_r)
                # Yr = re(X@Mr^T) - im(X@Mi^T); Yi = im(X@Mr^T) + re(X@Mi^T)
                c_t = sb.tile([128, ARW], fp, tag="c_t", name=f"c_t{c}", bufs=2)
                c3 = c_t.rearrange("p (t j) -> p t j", j=YTW)
                r3 = cc_r.rearrange("p (t j) -> p t j", j=128)
                i3 = tp_i.rearrange("p (t j) -> p t j", j=128)
                nc.vector.tensor_sub(
                    c3[:, :, 64:128], r3[:, :, 0:64], i3[:, :, 64:128]
                )
                nc.vector.tensor_add(
                    c3[:, :, 128:192], r3[:, :, 64:128], i3[:, :, 0:64]
                )
                nc.vector.tensor_scalar_mul(
                    c3[:, :, 0:64], c3[:, :, 128:192], -1.0
                )
                nc.gpsimd.dma_start(out=arins[c], in_=c_t)

                # rendezvous: tiny AR on the same (gpsimd) queue as the arin
                # write; its completion implies every core finished writing
                # arin[c], so the real AllReduce never reads a half-written
                # remote buffer.
                dbar_sb = sb.tile([1, 64], fp, tag="dbs", name=f"dbs{c}", bufs=2)
                nc.vector.tensor_copy(dbar_sb, c_t[0:1, 0:64])
                nc.gpsimd.dma_start(out=dbis[c], in_=dbar_sb)
                nc.gpsimd.collective_compute(
                    "AllReduce",
                    mybir.AluOpType.add,
                    ins=[dbis[c].opt()],
                    outs=[dbos[c].opt()],
                    replica_groups=[list(range(n_cores))],
                )
                nc.gpsimd.collective_compute(
                    "AllReduce",
                    mybir.AluOpType.add,
                    ins=[arins[c].opt()],
                    outs=[arouts[c].opt()],
                    replica_groups=[list(range(n_cores))],
                )

            # AR-dependent reads, all on the gpsimd queue after the chunk
            # loop: a late AR stalls nothing except the matmuls that truly
            # need it (the 6-deep bn buffer keeps the bulk queues streaming).
            yts = []
            for c in range(NKC):
                ytA = sbx.tile([128, ARW], fp, tag=f"ytA{c}", name=f"ytA{c}")
                nc.gpsimd.dma_start(out=ytA, in_=arouts[c])
                yts.append(ytA)

            # ---------------- mm2 (DMA-paced, right behind mt) ----------------
            # po banks continue the pA..pD tag rotations (all 8 PSUM banks).
            po = [
                ps.tile([128, 512], f32, tag=tg, name=f"po{h}", bufs=2)
                for h, tg in enumerate(
                    ("pA", "pA", "pB", "pB", "pC", "pC", "pD", "pD")
                )
            ]
            for kb in range(NKB):
                c = kb // KBC
                eng = nc.sync if kb % 2 == 0 else nc.scalar
                bn_t = sb.tile(
                    [128, 2 * dl], fp, tag="bnstream", name="bn", bufs=6
                )
                eng.dma_start(out=bn_t, in_=bn[kb * 128 : (kb + 1) * 128, :])
                s0 = (kb % KBC) * YTW
                st, sp = kb == 0, kb == NKB - 1
                for h in range(8):
                    nc.tensor.matmul(
                        po[h],
                        lhsT=yts[c][:, s0 + 64 : s0 + 192],
                        rhs=bn_t[:, h * 512 : (h + 1) * 512],
                        start=st,
                        stop=False,
                    )
                for h in range(8):
                    nc.tensor.matmul(
                        po[h],
                        lhsT=yts[c][:, s0 : s0 + 128],
                        rhs=bn_t[:, dl + h * 512 : dl + (h + 1) * 512],
                        start=False,
                        stop=sp,
                    )
            for h in range(8):
                o_t = sb.tile([128, 512], fp, tag="o_t", name="o_t", bufs=4)
                nc.vector.tensor_scalar_mul(o_t, po[h], 1.0 / (SCALE_M * SCALE_B))
                eng = nc.sync if h % 2 == 0 else nc.gpsimd
                eng.dma_start(out=out[:, h * 512 : (h + 1) * 512], in_=o_t)

    nc.compile()
    return nc


def _get_nc(n_cores=NCORES, k=K, dl=DL):
    key = (n_cores, k, dl)
    if key not in _nc_cache:
        _nc_cache[key] = build_nc(n_cores, k, dl)
    return _nc_cache[key]


def _prep_in_maps(X_re, X_im, bases_re, bases_im, weight_re, weight_im):
    cdt = np.float16
    f32 = np.float32
    X_re = np.asarray(X_re, f32)
    X_im = np.asarray(X_im, f32)
    bases_re = np.asarray(bases_re, f32)
    bases_im = np.asarray(bases_im, f32)
    wr = np.asarray(weight_re, f32)[:, None]
    wi = np.asarray(weight_im, f32)[:, None]

    # M = diag(w) @ conj(B): Mr = wr*Br + wi*Bi ; Mi = wi*Br - wr*Bi
    mr = (wr * bases_re + wi * bases_im) * np.float32(SCALE_M)
    mi = (wi * bases_re - wr * bases_im) * np.float32(SCALE_M)
    bsr = (bases_re * np.float32(SCALE_B)).astype(cdt)
    bsi = (bases_im * np.float32(SCALE_B)).astype(cdt)
    mr = mr.astype(cdt)
    mi = mi.astype(cdt)

    in_maps = []
    for c in range(NCORES):
        lo = c * DL
        hi = min((c + 1) * DL, D)
        n = hi - lo

        # xt[p, dt*128 + j] = Xstack^T[dt*128+p, j], j: 0:64 re, 64:128 im
        xtd = np.zeros((DL, 128), cdt)
        xtd[:n, 0:64] = X_re[:, lo:hi].T.astype(cdt)
        xtd[:n, 64:128] = X_im[:, lo:hi].T.astype(cdt)
        xt = (
            xtd.reshape(NDT, 128, 128).transpose(1, 0, 2).reshape(128, DL)
        )

        # mt slab (kc, s): rows p=d-within-tile, cols dtl*2*KCW + [Mr | Mi]
        # for k-chunk kc, d-tile dt = s*DPC + dtl.
        mrT = np.zeros((DL, K), cdt)
        miT = np.zeros((DL, K), cdt)
        mrT[:n, :] = mr[:, lo:hi].T
        miT[:n, :] = mi[:, lo:hi].T
        # r4[dt, p, kc, q]
        r4 = mrT.reshape(NDT, 128, NKC, KCW)
        i4 = miT.reshape(NDT, 128, NKC, KCW)
        mt = np.empty((NKC, MSLB, 128, DPC, 2, KCW), cdt)
        # -> [kc, s, p, dtl, plane, q]
        mt[:, :, :, :, 0, :] = (
            r4.transpose(2, 0, 1, 3)
            .reshape(NKC, MSLB, DPC, 128, KCW)
            .transpose(0, 1, 3, 2, 4)
        )
        mt[:, :, :, :, 1, :] = (
            i4.transpose(2, 0, 1, 3)
            .reshape(NKC, MSLB, DPC, 128, KCW)
            .transpose(0, 1, 3, 2, 4)
        )
        mt = mt.reshape(NKC * MSLB * 128, DPC * 2 * KCW)

        # bn[kb*128 + p, :] = [Br[k, d-shard] | Bi[k, d-shard]]
        bnd = np.zeros((K, 2 * DL), cdt)
        bnd[:, 0:n] = bsr[:, lo:hi]
        bnd[:, DL : DL + n] = bsi[:, lo:hi]

        in_maps.append({"xt": xt, "mt": mt, "bn": bnd})
    return in_maps


def run(inputs, trace=False, trace_kwargs=None):
    """Returns (full complex64 output [64, 32400], BassKernelResults)."""
    from concourse.bass_utils import run_bass_kernel_spmd

    in_maps = _prep_in_maps(**inputs)
    nc = _get_nc()
    res = run_bass_kernel_spmd(
        nc,
        in_maps,
        core_ids=list(range(NCORES)),
        trace=trace,
        **(trace_kwargs or {}),
    )
    parts = []
    for c in range(NCORES):
        o = res.results[c]["out"].astype(np.float32)
        parts.append(o[0:64, :] + 1j * o[64:128, :].astype(np.complex64))
    full = np.concatenate(parts, axis=1)[:, :D].astype(np.complex64)
    return full, res


def kernel(**inputs) -> np.ndarray:
    out, _ = run(inputs, trace=False)
    return out
